# revision 4
# baseline (speedup 1.0000x reference)
"""Sharded multi-head attention for TRN2 (8 NeuronCores).

Problem: B=4, H=16, S=2048, DK=64 attention with boolean mask [B,1,S,S]
(True entries masked out).  The 64 (batch, head) pairs are independent, so
we shard them across the 8 cores: core c handles batch c//2, heads
(c%2)*8 .. (c%2)*8+8.

Per-core algorithm (per head), all in transposed [k, q] orientation:
  - scores_T[k, q] = sum_d K[k,d] Q[q,d]      (PE matmul, bf16, lhsT=K^T tile)
  - w = exp(scores_T * 1/8)                    (ACT, reads PSUM, no max-sub
                                                needed: scores ~ N(0,1))
  - w *= keep_T[k, q]                          (DVE bf16 tensor_tensor 2x)
  - acc[v, q] += V'[k, v]^T w                  (PE matmul, V' = [V | ones]
                                                so row 64 of acc = softmax sums)
  - epilogue: transpose acc 128-q-slices to [q, 65], recip of col 64,
    scale cols 0..63, DMA out.
"""

import numpy as np
import ml_dtypes
from contextlib import ExitStack

import concourse.bass as bass
import concourse.tile as tile
from concourse import bacc, mybir
from concourse.bass_utils import run_bass_kernel_spmd
from concourse.masks import make_identity

B, H, S, DK = 4, 16, 2048, 64
N_CORES = 8
HPC = (B * H) // N_CORES  # heads per core = 8

P = 128            # k-tile size / partition count
NKT = S // P       # 16 k tiles
QCH = 1024         # q chunk (scores free size; 2 PSUM banks)
NQ = S // QCH      # 2 q chunks
MM_N = 512         # matmul free dim (one PSUM bank)

BF16 = mybir.dt.bfloat16
F32 = mybir.dt.float32


def build_nc():
    nc = bacc.Bacc(None, target_bir_lowering=False)
    q_ext = nc.declare_dram_parameter("q", [HPC, S, DK], BF16, isOutput=False)
    k_ext = nc.declare_dram_parameter("k", [HPC, S, DK], BF16, isOutput=False)
    v_ext = nc.declare_dram_parameter("v", [HPC, S, DK], BF16, isOutput=False)
    keep_ext = nc.declare_dram_parameter("keep", [S, S], BF16, isOutput=False)
    out_ext = nc.declare_dram_parameter("out", [HPC, S, DK], F32, isOutput=True)

    with tile.TileContext(nc) as tc, ExitStack() as ctx:
        singles = ctx.enter_context(tc.tile_pool(name="singles", bufs=1))
        qk_pool = ctx.enter_context(tc.tile_pool(name="qk", bufs=2))
        v_pool = ctx.enter_context(tc.tile_pool(name="vp", bufs=2))
        w_pool = ctx.enter_context(tc.tile_pool(name="wp", bufs=3))
        ep_pool = ctx.enter_context(tc.tile_pool(name="ep", bufs=2))
        o_pool = ctx.enter_context(tc.tile_pool(name="op", bufs=4))
        sc_ps = ctx.enter_context(tc.tile_pool(name="scps", bufs=2, space="PSUM"))
        acc_ps = ctx.enter_context(tc.tile_pool(name="accps", bufs=1, space="PSUM"))
        tr_ps = ctx.enter_context(tc.tile_pool(name="trps", bufs=2, space="PSUM"))

        identity = singles.tile([P, P], BF16)
        make_identity(nc, identity)

        # Resident mask: keep_sb[p, t, q] = keep[t*128 + p, q]
        keep_sb = singles.tile([P, NKT, S], BF16)
        nc.sync.dma_start(out=keep_sb, in_=keep_ext.rearrange("(t p) q -> p t q", p=P))

        for h in range(HPC):
            qT = qk_pool.tile([DK, S], BF16, tag="qT")
            kT = qk_pool.tile([DK, S], BF16, tag="kT")
            nc.sync.dma_start_transpose(qT, q_ext[h])
            nc.sync.dma_start_transpose(kT, k_ext[h])

            vp = v_pool.tile([P, NKT, DK + 1], BF16, tag="vp")
            nc.vector.memset(vp[:, :, DK : DK + 1], 1.0)
            nc.sync.dma_start(
                out=vp[:, :, 0:DK], in_=v_ext[h].rearrange("(t p) d -> p t d", p=P)
            )

            for qc in range(NQ):
                q0 = qc * QCH
                acc = acc_ps.tile([DK + 1, QCH], F32, tag="acc")
                for kt in range(NKT):
                    sc = sc_ps.tile([P, QCH], F32, tag="sc")
                    for j in range(QCH // MM_N):
                        nc.tensor.matmul(
                            sc[:, j * MM_N : (j + 1) * MM_N],
                            kT[:, kt * P : (kt + 1) * P],
                            qT[:, q0 + j * MM_N : q0 + (j + 1) * MM_N],
                            start=True,
                            stop=True,
                        )
                    w = w_pool.tile([P, QCH], BF16, tag="w")
                    nc.scalar.activation(
                        w, sc, mybir.ActivationFunctionType.Exp, scale=0.125
                    )
                    nc.vector.tensor_mul(w, w, keep_sb[:, kt, q0 : q0 + QCH])
                    for j in range(QCH // MM_N):
                        nc.tensor.matmul(
                            acc[:, j * MM_N : (j + 1) * MM_N],
                            vp[:, kt, :],
                            w[:, j * MM_N : (j + 1) * MM_N],
                            start=(kt == 0),
                            stop=(kt == NKT - 1),
                        )

                accT = ep_pool.tile([DK + 1, QCH], BF16, tag="accT")
                nc.vector.tensor_copy(accT, acc)
                for j in range(QCH // P):
                    tr = tr_ps.tile([P, DK + 1], BF16, tag="tr")
                    nc.tensor.transpose(
                        tr, accT[:, j * P : (j + 1) * P], identity[0 : DK + 1, 0 : DK + 1]
                    )
                    recip = o_pool.tile([P, 1], F32, tag="recip")
                    nc.vector.reciprocal(recip, tr[:, DK : DK + 1])
                    ot = o_pool.tile([P, DK], F32, tag="ot")
                    nc.vector.tensor_scalar_mul(ot, tr[:, 0:DK], recip)
                    nc.sync.dma_start(
                        out=out_ext[h, q0 + j * P : q0 + (j + 1) * P, :], in_=ot
                    )
    nc.finalize()
    return nc


_NC_CACHE = {}


def get_nc():
    if "nc" not in _NC_CACHE:
        _NC_CACHE["nc"] = build_nc()
    return _NC_CACHE["nc"]


def kernel(Q, K, V, mask, _trace=False, _tmpdir=None):
    Qb = np.asarray(Q, dtype=np.float32).astype(ml_dtypes.bfloat16)
    Kb = np.asarray(K, dtype=np.float32).astype(ml_dtypes.bfloat16)
    Vb = np.asarray(V, dtype=np.float32).astype(ml_dtypes.bfloat16)
    # keep_T[b, k, q] = not mask[b, 0, q, k]
    keep = (~np.asarray(mask)[:, 0]).transpose(0, 2, 1)
    keep = np.ascontiguousarray(keep).astype(ml_dtypes.bfloat16)

    in_maps = []
    for c in range(N_CORES):
        b, h0 = c // 2, (c % 2) * HPC
        in_maps.append(
            {
                "q": np.ascontiguousarray(Qb[b, h0 : h0 + HPC]),
                "k": np.ascontiguousarray(Kb[b, h0 : h0 + HPC]),
                "v": np.ascontiguousarray(Vb[b, h0 : h0 + HPC]),
                "keep": keep[b],
            }
        )

    nc = get_nc()
    res = run_bass_kernel_spmd(
        nc, in_maps, core_ids=list(range(N_CORES)), trace=_trace, tmpdir=_tmpdir
    )
    out = np.empty((B, H, S, DK), np.float32)
    for c in range(N_CORES):
        b, h0 = c // 2, (c % 2) * HPC
        out[b, h0 : h0 + HPC] = np.asarray(res.results[c]["out"], np.float32)
    if _trace:
        return out, res
    return out


# revision 7
# speedup vs baseline: 4.3748x; 4.3748x over previous
"""Sharded multi-head attention for TRN2 (8 NeuronCores).

Problem: B=4, H=16, S=2048, DK=64 attention with boolean mask [B,1,S,S]
(True entries masked out).  The 64 (batch, head) pairs are independent:
core c handles batch c//2, heads (c%2)*8 .. (c%2)*8+8.

Per-core algorithm (per head), in transposed [k, q] orientation:
  - scores_T[k, q] = sum_d K[k,d] Q[q,d]      (PE, bf16, lhsT = K^T tile)
  - w = exp(scores_T / 8)                      (ACT from PSUM, no max-sub:
                                                scores ~ N(0,1))
  - w *= keep_T[k, q]                          (DVE bf16 tensor_tensor)
  - acc[v, q] += V'[k, v]^T w                  (PE, V' = [V | ones] so row 64
                                                of acc = softmax denominators)
  - epilogue: recip of row 64, rank-1 PE broadcast to 64 partitions,
    multiply, DMA out in [d, q] layout (host un-transposes the view).

All DMAs are partition-major with >=2KB contiguous runs (host pre-swizzles
inputs); descriptor counts stay ~3K total vs 133K for the naive layout.
"""

import numpy as np
import ml_dtypes
from contextlib import ExitStack

import concourse.bass as bass
import concourse.tile as tile
from concourse import bacc, mybir
from concourse.bass_utils import run_bass_kernel_spmd

B, H, S, DK = 4, 16, 2048, 64
N_CORES = 8
HPC = (B * H) // N_CORES  # heads per core = 8

P = 128            # k-tile size / partition count
NKT = S // P       # 16 k tiles
QCH = 1024         # q chunk (scores tile free size; 2 PSUM banks)
NQ = S // QCH      # 2 q chunks
MM_N = 512         # matmul free dim (one PSUM bank)

BF16 = mybir.dt.bfloat16
F32 = mybir.dt.float32
BF = ml_dtypes.bfloat16


def build_nc():
    nc = bacc.Bacc(None, target_bir_lowering=False)
    # qkt[h] = [Q[h]^T ; K[h]^T]  ([0:64] = d-of-Q, [64:128] = d-of-K)
    qkt_ext = nc.declare_dram_parameter("qkt", [HPC, P, S], BF16, isOutput=False)
    # vp[h, p, t, :] = [V[h, t*128+p, :], 1.0]
    vp_ext = nc.declare_dram_parameter("vp", [HPC, P, NKT, DK + 1], BF16, isOutput=False)
    # keep[p, t, q] = not mask[q, t*128+p]
    keep_ext = nc.declare_dram_parameter("keep", [P, NKT, S], BF16, isOutput=False)
    # out_T[h, d, q] = attention output transposed (host un-transposes)
    out_ext = nc.declare_dram_parameter("outT", [HPC, DK, S], F32, isOutput=True)

    with tile.TileContext(nc) as tc, ExitStack() as ctx:
        singles = ctx.enter_context(tc.tile_pool(name="singles", bufs=1))
        qk_pool = ctx.enter_context(tc.tile_pool(name="qk", bufs=2))
        v_pool = ctx.enter_context(tc.tile_pool(name="vpool", bufs=2))
        w_pool = ctx.enter_context(tc.tile_pool(name="wp", bufs=3))
        ep_pool = ctx.enter_context(tc.tile_pool(name="ep", bufs=2))
        sc_ps = ctx.enter_context(tc.tile_pool(name="scps", bufs=2, space="PSUM"))
        acc_ps = ctx.enter_context(tc.tile_pool(name="accps", bufs=2, space="PSUM"))

        ones64 = singles.tile([1, DK], BF16)
        nc.vector.memset(ones64, 1.0)

        keep_sb = singles.tile([P, NKT, S], BF16)
        nc.sync.dma_start(out=keep_sb, in_=keep_ext[:])

        for h in range(HPC):
            qT = qk_pool.tile([DK, S], BF16, tag="qT")
            kT = qk_pool.tile([DK, S], BF16, tag="kT")
            nc.sync.dma_start(out=qT, in_=qkt_ext[h, 0:DK])
            nc.sync.dma_start(out=kT, in_=qkt_ext[h, DK : 2 * DK])

            vp = v_pool.tile([P, NKT, DK + 1], BF16, tag="vp")
            nc.sync.dma_start(out=vp, in_=vp_ext[h])

            for qc in range(NQ):
                q0 = qc * QCH
                acc = acc_ps.tile([DK + 1, QCH], F32, tag="acc")
                for kt in range(NKT):
                    sc = sc_ps.tile([P, QCH], F32, tag="sc")
                    for j in range(QCH // MM_N):
                        nc.tensor.matmul(
                            sc[:, j * MM_N : (j + 1) * MM_N],
                            kT[:, kt * P : (kt + 1) * P],
                            qT[:, q0 + j * MM_N : q0 + (j + 1) * MM_N],
                            start=True,
                            stop=True,
                        )
                    w = w_pool.tile([P, QCH], BF16, tag="w")
                    nc.scalar.activation(
                        w, sc, mybir.ActivationFunctionType.Exp, scale=0.125
                    )
                    nc.vector.tensor_mul(w, w, keep_sb[:, kt, q0 : q0 + QCH])
                    for j in range(QCH // MM_N):
                        nc.tensor.matmul(
                            acc[:, j * MM_N : (j + 1) * MM_N],
                            vp[:, kt, :],
                            w[:, j * MM_N : (j + 1) * MM_N],
                            start=(kt == 0),
                            stop=(kt == NKT - 1),
                        )

                # epilogue: normalize in [d, q] layout
                accS = ep_pool.tile([DK + 1, QCH], BF16, tag="accS")
                nc.vector.tensor_copy(accS, acc)
                recipF = ep_pool.tile([1, QCH], F32, tag="recipF")
                nc.vector.reciprocal(recipF, acc[DK : DK + 1, :])
                recipS = ep_pool.tile([1, QCH], BF16, tag="recipS")
                nc.vector.tensor_copy(recipS, recipF)
                # rank-1 broadcast: bc[0:64, q] = ones64^T @ recipS
                bc = sc_ps.tile([DK, QCH], F32, tag="sc")
                for j in range(QCH // MM_N):
                    nc.tensor.matmul(
                        bc[:, j * MM_N : (j + 1) * MM_N],
                        ones64,
                        recipS[:, j * MM_N : (j + 1) * MM_N],
                        start=True,
                        stop=True,
                    )
                bcS = ep_pool.tile([DK, QCH], BF16, tag="bcS")
                nc.vector.tensor_copy(bcS, bc)
                outf = ep_pool.tile([DK, QCH], F32, tag="outf")
                nc.vector.tensor_mul(outf, accS[0:DK], bcS)
                nc.gpsimd.dma_start(out=out_ext[h, :, q0 : q0 + QCH], in_=outf)
    nc.finalize()
    return nc


_NC_CACHE = {}


def get_nc():
    if "nc" not in _NC_CACHE:
        _NC_CACHE["nc"] = build_nc()
    return _NC_CACHE["nc"]


def kernel(Q, K, V, mask, _trace=False, _tmpdir=None):
    Q = np.asarray(Q, dtype=np.float32)
    K = np.asarray(K, dtype=np.float32)
    V = np.asarray(V, dtype=np.float32)
    mask = np.asarray(mask)

    in_maps = []
    for c in range(N_CORES):
        b, h0 = c // 2, (c % 2) * HPC
        qkt = np.empty((HPC, P, S), BF)
        qkt[:, 0:DK] = Q[b, h0 : h0 + HPC].transpose(0, 2, 1)
        qkt[:, DK : 2 * DK] = K[b, h0 : h0 + HPC].transpose(0, 2, 1)
        vp = np.empty((HPC, P, NKT, DK + 1), BF)
        # [h, t*128+p, d] -> [h, p, t, d]
        vp[:, :, :, 0:DK] = (
            V[b, h0 : h0 + HPC].reshape(HPC, NKT, P, DK).transpose(0, 2, 1, 3)
        )
        vp[:, :, :, DK] = 1.0
        if c % 2 == 0:
            # keep shared across the 2 cores of a batch
            kp = (~mask[b, 0]).T  # [k, q]
            keep = np.ascontiguousarray(
                kp.reshape(NKT, P, S).transpose(1, 0, 2)
            ).astype(BF)
        in_maps.append({"qkt": qkt, "vp": vp, "keep": keep})

    nc = get_nc()
    res = run_bass_kernel_spmd(
        nc, in_maps, core_ids=list(range(N_CORES)), trace=_trace, tmpdir=_tmpdir
    )
    out = np.empty((B, H, S, DK), np.float32)
    for c in range(N_CORES):
        b, h0 = c // 2, (c % 2) * HPC
        # res: [HPC, DK, S] -> [HPC, S, DK]
        out[b, h0 : h0 + HPC] = np.asarray(res.results[c]["outT"]).transpose(0, 2, 1)
    if _trace:
        return out, res
    return out


# revision 10
# speedup vs baseline: 7.0592x; 1.6136x over previous
"""Sharded multi-head attention for TRN2 (8 NeuronCores).

Problem: B=4, H=16, S=2048, DK=64 attention with boolean mask [B,1,S,S]
(True entries masked out).  The 64 (batch, head) pairs are independent:
core c handles batch c//2, heads (c%2)*8 .. (c%2)*8+8.

Per-core algorithm, heads processed in PAIRS (A, B) sharing the PE array:
  - scores_T[k, q] for A and B run CONCURRENTLY in PE row-groups [0:64] /
    [64:128] (contraction dim d=64 each, tile_position packing).
  - one ACT exp over the pair's [128, 1024] PSUM tile (scale=1/8 folded,
    no max-subtraction: scores ~ N(0,1)).
  - mask multiply on DVE (bf16 2x), keep_T shared across heads.
  - PV: acc[v, q] += V'[k, v]^T w per head, V' = [V | ones] so row 64 of
    acc accumulates the softmax denominators.
  - epilogue per (head, q-chunk): reciprocal_approx_fast of row 64,
    rank-1 PE broadcast to 64 partitions, multiply, DMA out in [d, q]
    layout (host un-transposes the view; pure layout, no host math).

All DMAs are partition-major with >=2KB contiguous runs (host pre-swizzles
inputs, ones column baked into V').
"""

import numpy as np
import ml_dtypes
from contextlib import ExitStack

import concourse.bass as bass
import concourse.tile as tile
from concourse import bacc, mybir
from concourse.bass_utils import run_bass_kernel_spmd

B, H, S, DK = 4, 16, 2048, 64
N_CORES = 8
HPC = (B * H) // N_CORES  # heads per core = 8
NPAIR = HPC // 2

P = 128            # k-tile size / partition count
NKT = S // P       # 16 k tiles
QCH = 512          # q chunk per head (pair tile = [128, 1024] = 2 PSUM banks)
NQ = S // QCH      # 4 q chunks

BF16 = mybir.dt.bfloat16
F32 = mybir.dt.float32
BF = ml_dtypes.bfloat16


def build_nc():
    nc = bacc.Bacc(None, target_bir_lowering=False)
    # qkt[pair, 0] = [Q_A^T ; Q_B^T] stacked on partitions, [pair, 1] = K
    qkt_ext = nc.declare_dram_parameter("qkt", [NPAIR, 2, P, S], BF16, isOutput=False)
    # vp[h, p, t, :] = [V[h, t*128+p, :], 1.0]
    vp_ext = nc.declare_dram_parameter("vp", [HPC, P, NKT, DK + 1], BF16, isOutput=False)
    # keep[p, t, q] = not mask[q, t*128+p]
    keep_ext = nc.declare_dram_parameter("keep", [P, NKT, S], BF16, isOutput=False)
    # out_T[h, d, q] (host un-transposes)
    out_ext = nc.declare_dram_parameter("outT", [HPC, DK, S], F32, isOutput=True)

    with tile.TileContext(nc) as tc, ExitStack() as ctx:
        singles = ctx.enter_context(tc.tile_pool(name="singles", bufs=1))
        qk_pool = ctx.enter_context(tc.tile_pool(name="qk", bufs=2))
        v_pool = ctx.enter_context(tc.tile_pool(name="vpool", bufs=2))
        w_pool = ctx.enter_context(tc.tile_pool(name="wp", bufs=3))
        ep_pool = ctx.enter_context(tc.tile_pool(name="ep", bufs=2))
        sc_ps = ctx.enter_context(tc.tile_pool(name="scps", bufs=2, space="PSUM"))
        acc_ps = ctx.enter_context(tc.tile_pool(name="accps", bufs=2, space="PSUM"))

        ones64 = singles.tile([1, DK], BF16)
        nc.vector.memset(ones64, 1.0)

        keep_sb = singles.tile([P, NKT, S], BF16)
        nc.sync.dma_start(out=keep_sb, in_=keep_ext[:])

        for pair in range(NPAIR):
            hA, hB = 2 * pair, 2 * pair + 1
            qT2 = qk_pool.tile([P, S], BF16, tag="qT2")
            kT2 = qk_pool.tile([P, S], BF16, tag="kT2")
            nc.sync.dma_start(out=qT2, in_=qkt_ext[pair, 0])
            nc.sync.dma_start(out=kT2, in_=qkt_ext[pair, 1])
            vpA = v_pool.tile([P, NKT, DK + 1], BF16, tag="vpA")
            vpB = v_pool.tile([P, NKT, DK + 1], BF16, tag="vpB")
            nc.sync.dma_start(out=vpA, in_=vp_ext[hA])
            nc.sync.dma_start(out=vpB, in_=vp_ext[hB])

            for qc in range(NQ):
                q0 = qc * QCH
                accA = acc_ps.tile([DK + 1, QCH], F32, tag="accA")
                accB = acc_ps.tile([DK + 1, QCH], F32, tag="accB")
                for kt in range(NKT):
                    k0 = kt * P
                    sc = sc_ps.tile([P, 2 * QCH], F32, tag="sc")
                    nc.tensor.matmul(
                        sc[:, 0:QCH],
                        kT2[0:DK, k0 : k0 + P],
                        qT2[0:DK, q0 : q0 + QCH],
                        start=True,
                        stop=True,
                        tile_position=(0, 0),
                    )
                    nc.tensor.matmul(
                        sc[:, QCH : 2 * QCH],
                        kT2[DK : 2 * DK, k0 : k0 + P],
                        qT2[DK : 2 * DK, q0 : q0 + QCH],
                        start=True,
                        stop=True,
                        tile_position=(64, 0),
                    )
                    w = w_pool.tile([P, 2 * QCH], BF16, tag="w")
                    nc.scalar.activation(
                        w, sc, mybir.ActivationFunctionType.Exp, scale=0.125
                    )
                    keep_slice = keep_sb[:, kt, q0 : q0 + QCH]
                    nc.vector.tensor_mul(w[:, 0:QCH], w[:, 0:QCH], keep_slice)
                    nc.vector.tensor_mul(
                        w[:, QCH : 2 * QCH], w[:, QCH : 2 * QCH], keep_slice
                    )
                    nc.tensor.matmul(
                        accA,
                        vpA[:, kt],
                        w[:, 0:QCH],
                        start=(kt == 0),
                        stop=(kt == NKT - 1),
                    )
                    nc.tensor.matmul(
                        accB,
                        vpB[:, kt],
                        w[:, QCH : 2 * QCH],
                        start=(kt == 0),
                        stop=(kt == NKT - 1),
                    )

                for h, acc in ((hA, accA), (hB, accB)):
                    accS = ep_pool.tile([DK + 1, QCH], BF16, tag="accS")
                    nc.vector.tensor_copy(accS, acc)
                    rowF = ep_pool.tile([1, QCH], F32, tag="rowF")
                    nc.vector.tensor_copy(rowF, acc[DK : DK + 1, :])
                    recipF = ep_pool.tile([1, QCH], F32, tag="recipF")
                    nc.vector.reciprocal_approx_fast(recipF, rowF)
                    recipS = ep_pool.tile([1, QCH], BF16, tag="recipS")
                    nc.vector.tensor_copy(recipS, recipF)
                    bc = sc_ps.tile([DK, QCH], F32, tag="sc")
                    nc.tensor.matmul(bc, ones64, recipS, start=True, stop=True)
                    bcS = ep_pool.tile([DK, QCH], BF16, tag="bcS")
                    nc.vector.tensor_copy(bcS, bc)
                    outf = ep_pool.tile([DK, QCH], F32, tag="outf")
                    nc.vector.tensor_mul(outf, accS[0:DK], bcS)
                    nc.gpsimd.dma_start(out=out_ext[h, :, q0 : q0 + QCH], in_=outf)
    nc.finalize()
    return nc


_NC_CACHE = {}


def get_nc():
    if "nc" not in _NC_CACHE:
        _NC_CACHE["nc"] = build_nc()
    return _NC_CACHE["nc"]


def kernel(Q, K, V, mask, _trace=False, _tmpdir=None):
    Q = np.asarray(Q, dtype=np.float32)
    K = np.asarray(K, dtype=np.float32)
    V = np.asarray(V, dtype=np.float32)
    mask = np.asarray(mask)

    in_maps = []
    for c in range(N_CORES):
        b, h0 = c // 2, (c % 2) * HPC
        # [pair, {q,k}, 128, S]: partitions 0:64 = head A dims, 64:128 = head B
        qkt = np.empty((NPAIR, 2, P, S), BF)
        qt = Q[b, h0 : h0 + HPC].transpose(0, 2, 1).reshape(NPAIR, 2 * DK, S)
        kt = K[b, h0 : h0 + HPC].transpose(0, 2, 1).reshape(NPAIR, 2 * DK, S)
        qkt[:, 0] = qt
        qkt[:, 1] = kt
        vp = np.empty((HPC, P, NKT, DK + 1), BF)
        vp[:, :, :, 0:DK] = (
            V[b, h0 : h0 + HPC].reshape(HPC, NKT, P, DK).transpose(0, 2, 1, 3)
        )
        vp[:, :, :, DK] = 1.0
        if c % 2 == 0:
            kp = (~mask[b, 0]).T  # [k, q]
            keep = np.ascontiguousarray(
                kp.reshape(NKT, P, S).transpose(1, 0, 2)
            ).astype(BF)
        in_maps.append({"qkt": qkt, "vp": vp, "keep": keep})

    nc = get_nc()
    res = run_bass_kernel_spmd(
        nc, in_maps, core_ids=list(range(N_CORES)), trace=_trace, tmpdir=_tmpdir
    )
    out = np.empty((B, H, S, DK), np.float32)
    for c in range(N_CORES):
        b, h0 = c // 2, (c % 2) * HPC
        out[b, h0 : h0 + HPC] = np.asarray(res.results[c]["outT"]).transpose(0, 2, 1)
    if _trace:
        return out, res
    return out


# revision 13
# speedup vs baseline: 7.3257x; 1.0378x over previous
"""Sharded multi-head attention for TRN2 (8 NeuronCores).

Problem: B=4, H=16, S=2048, DK=64 attention with boolean mask [B,1,S,S]
(True entries masked out).  The 64 (batch, head) pairs are independent:
core c handles batch c//2, heads (c%2)*8 .. (c%2)*8+8.

Per-core algorithm, heads processed in PAIRS (A, B) sharing the PE array:
  - scores_T[k, q] for A and B run CONCURRENTLY in PE row-groups [0:64] /
    [64:128] (contraction dim d=64 each, tile_position packing).
  - one ACT exp over the pair's [128, 1024] PSUM tile (scale=1/8 folded,
    no max-subtraction: scores ~ N(0,1)).
  - mask multiply on DVE (bf16 2x), keep_T shared across heads.
  - PV: acc[v, q] += V'[k, v]^T w per head, V' = [V | ones] so row 64 of
    acc accumulates the softmax denominators.
  - epilogue per (head, q-chunk): reciprocal_approx_fast of row 64,
    rank-1 PE broadcast to 64 partitions, multiply, DMA out in [d, q]
    layout (host un-transposes the view; pure layout, no host math).

All DMAs are partition-major with >=2KB contiguous runs (host pre-swizzles
inputs, ones column baked into V').
"""

import numpy as np
import ml_dtypes
from contextlib import ExitStack

import concourse.bass as bass
import concourse.tile as tile
from concourse import bacc, mybir
from concourse.bass_utils import run_bass_kernel_spmd

B, H, S, DK = 4, 16, 2048, 64
N_CORES = 8
HPC = (B * H) // N_CORES  # heads per core = 8
NPAIR = HPC // 2

P = 128            # k-tile size / partition count
NKT = S // P       # 16 k tiles
QCH = 512          # q chunk per head (pair tile = [128, 1024] = 2 PSUM banks)
NQ = S // QCH      # 4 q chunks

BF16 = mybir.dt.bfloat16
F32 = mybir.dt.float32
BF = ml_dtypes.bfloat16


def build_nc():
    nc = bacc.Bacc(None, target_bir_lowering=False)
    # qkt[pair, 0] = [Q_A^T ; Q_B^T] stacked on partitions, [pair, 1] = K
    qkt_ext = nc.declare_dram_parameter("qkt", [NPAIR, 2, P, S], BF16, isOutput=False)
    # vp[h, p, t, :] = [V[h, t*128+p, :], 1.0]
    vp_ext = nc.declare_dram_parameter("vp", [HPC, P, NKT, DK + 1], BF16, isOutput=False)
    # keep[p, t, q] = not mask[q, t*128+p]
    keep_ext = nc.declare_dram_parameter("keep", [P, NKT, S], BF16, isOutput=False)
    # out_T[h, d, q] (host un-transposes)
    out_ext = nc.declare_dram_parameter("outT", [HPC, DK, S], F32, isOutput=True)

    with tile.TileContext(nc) as tc, ExitStack() as ctx:
        singles = ctx.enter_context(tc.tile_pool(name="singles", bufs=1))
        qk_pool = ctx.enter_context(tc.tile_pool(name="qk", bufs=2))
        v_pool = ctx.enter_context(tc.tile_pool(name="vpool", bufs=2))
        w_pool = ctx.enter_context(tc.tile_pool(name="wp", bufs=4))
        ep_pool = ctx.enter_context(tc.tile_pool(name="ep", bufs=2))
        sc_ps = ctx.enter_context(tc.tile_pool(name="scps", bufs=2, space="PSUM"))
        acc_ps = ctx.enter_context(tc.tile_pool(name="accps", bufs=2, space="PSUM"))

        ones64 = singles.tile([1, DK], BF16)
        nc.vector.memset(ones64, 1.0)

        keep_sb = singles.tile([P, NKT, S], BF16)
        nc.sync.dma_start(out=keep_sb, in_=keep_ext[:])

        for pair in range(NPAIR):
            hA, hB = 2 * pair, 2 * pair + 1
            qT2 = qk_pool.tile([P, S], BF16, tag="qT2")
            kT2 = qk_pool.tile([P, S], BF16, tag="kT2")
            nc.sync.dma_start(out=qT2, in_=qkt_ext[pair, 0])
            nc.sync.dma_start(out=kT2, in_=qkt_ext[pair, 1])
            vpA = v_pool.tile([P, NKT, DK + 1], BF16, tag="vpA")
            vpB = v_pool.tile([P, NKT, DK + 1], BF16, tag="vpB")
            nc.sync.dma_start(out=vpA, in_=vp_ext[hA])
            nc.sync.dma_start(out=vpB, in_=vp_ext[hB])

            for qc in range(NQ):
                q0 = qc * QCH
                accA = acc_ps.tile([DK + 1, QCH], F32, tag="accA")
                accB = acc_ps.tile([DK + 1, QCH], F32, tag="accB")
                for kt in range(NKT):
                    k0 = kt * P
                    sc = sc_ps.tile([P, 2 * QCH], F32, tag="sc")
                    nc.tensor.matmul(
                        sc[:, 0:QCH],
                        kT2[0:DK, k0 : k0 + P],
                        qT2[0:DK, q0 : q0 + QCH],
                        start=True,
                        stop=True,
                        tile_position=(0, 0),
                    )
                    nc.tensor.matmul(
                        sc[:, QCH : 2 * QCH],
                        kT2[DK : 2 * DK, k0 : k0 + P],
                        qT2[DK : 2 * DK, q0 : q0 + QCH],
                        start=True,
                        stop=True,
                        tile_position=(64, 0),
                    )
                    w = w_pool.tile([P, 2 * QCH], BF16, tag="w")
                    nc.scalar.activation(
                        w, sc, mybir.ActivationFunctionType.Exp, scale=0.125
                    )
                    # one masked multiply over both heads: keep slice is
                    # broadcast (stride-0) over the head dim
                    keep_slice = keep_sb[:, kt, q0 : q0 + QCH]
                    keep2 = bass.AP(
                        tensor=keep_slice.tensor,
                        offset=keep_slice.offset,
                        ap=[keep_slice.ap[0], [0, 2], keep_slice.ap[1]],
                    )
                    w2 = w.rearrange("p (r q) -> p r q", r=2)
                    nc.vector.tensor_mul(w2, w2, keep2)
                    nc.tensor.matmul(
                        accA,
                        vpA[:, kt],
                        w[:, 0:QCH],
                        start=(kt == 0),
                        stop=(kt == NKT - 1),
                    )
                    nc.tensor.matmul(
                        accB,
                        vpB[:, kt],
                        w[:, QCH : 2 * QCH],
                        start=(kt == 0),
                        stop=(kt == NKT - 1),
                    )

                for h, acc in ((hA, accA), (hB, accB)):
                    accS = ep_pool.tile([DK + 1, QCH], BF16, tag="accS")
                    nc.scalar.copy(accS, acc)
                    rowF = ep_pool.tile([1, QCH], F32, tag="rowF")
                    nc.vector.tensor_copy(rowF, acc[DK : DK + 1, :])
                    recipF = ep_pool.tile([1, QCH], F32, tag="recipF")
                    nc.vector.reciprocal_approx_fast(recipF, rowF)
                    recipS = ep_pool.tile([1, QCH], BF16, tag="recipS")
                    nc.vector.tensor_copy(recipS, recipF)
                    bc = sc_ps.tile([DK, QCH], F32, tag="sc")
                    nc.tensor.matmul(bc, ones64, recipS, start=True, stop=True)
                    outf = ep_pool.tile([DK, QCH], F32, tag="outf")
                    nc.vector.tensor_mul(outf, accS[0:DK], bc)
                    nc.gpsimd.dma_start(out=out_ext[h, :, q0 : q0 + QCH], in_=outf)
    nc.finalize()
    return nc


_NC_CACHE = {}


def get_nc():
    if "nc" not in _NC_CACHE:
        _NC_CACHE["nc"] = build_nc()
    return _NC_CACHE["nc"]


def kernel(Q, K, V, mask, _trace=False, _tmpdir=None):
    Q = np.asarray(Q, dtype=np.float32)
    K = np.asarray(K, dtype=np.float32)
    V = np.asarray(V, dtype=np.float32)
    mask = np.asarray(mask)

    in_maps = []
    for c in range(N_CORES):
        b, h0 = c // 2, (c % 2) * HPC
        # [pair, {q,k}, 128, S]: partitions 0:64 = head A dims, 64:128 = head B
        qkt = np.empty((NPAIR, 2, P, S), BF)
        qt = Q[b, h0 : h0 + HPC].transpose(0, 2, 1).reshape(NPAIR, 2 * DK, S)
        kt = K[b, h0 : h0 + HPC].transpose(0, 2, 1).reshape(NPAIR, 2 * DK, S)
        qkt[:, 0] = qt
        qkt[:, 1] = kt
        vp = np.empty((HPC, P, NKT, DK + 1), BF)
        vp[:, :, :, 0:DK] = (
            V[b, h0 : h0 + HPC].reshape(HPC, NKT, P, DK).transpose(0, 2, 1, 3)
        )
        vp[:, :, :, DK] = 1.0
        if c % 2 == 0:
            kp = (~mask[b, 0]).T  # [k, q]
            keep = np.ascontiguousarray(
                kp.reshape(NKT, P, S).transpose(1, 0, 2)
            ).astype(BF)
        in_maps.append({"qkt": qkt, "vp": vp, "keep": keep})

    nc = get_nc()
    res = run_bass_kernel_spmd(
        nc, in_maps, core_ids=list(range(N_CORES)), trace=_trace, tmpdir=_tmpdir
    )
    out = np.empty((B, H, S, DK), np.float32)
    for c in range(N_CORES):
        b, h0 = c // 2, (c % 2) * HPC
        out[b, h0 : h0 + HPC] = np.asarray(res.results[c]["outT"]).transpose(0, 2, 1)
    if _trace:
        return out, res
    return out
